# revision 1
# baseline (speedup 1.0000x reference)
# DecoderRNN loss kernel for 8 TRN2 NeuronCores.
#
# Strategy:
#   - Vocab-tensor-parallel output projection: core c owns out_w rows
#     [c*4000, (c+1)*4000), resident in SBUF as bf16.
#   - GRU recurrence replicated on every core (latency-bound; 63 serial steps).
#   - gi = x @ w_ih^T for all tokens precomputed as a batched matmul (phase B),
#     streamed per-step through a DRAM scratch buffer.
#   - sigmoid computed via tanh (sigma(x) = 0.5 + 0.5*tanh(x/2)) so the scalar
#     engine stays on one activation-table set (tanh+exp live together).
#   - The sent-GRU + EOS blend is only emitted for steps where the host sees
#     an EOS in next_words (host knows targets; EOS events are rare).
#   - Each core outputs per-token partials: sum(exp(logit-20)) over its vocab
#     shard and the target logit (0 if the target is not in its shard).
#     Host combines in float64: loss = -sum(valid * (tgt_logit - logsumexp)).
import numpy as np
import ml_dtypes

import concourse.bass as bass
import bass_rust
import concourse.tile as tile
import concourse.mybir as mybir
from concourse.bass_utils import run_bass_kernel_spmd

BF16NP = ml_dtypes.bfloat16
F32 = mybir.dt.float32
BF = mybir.dt.bfloat16
AF = mybir.ActivationFunctionType
OP = mybir.AluOpType

B, S, V, H = 32, 64, 32000, 512
T = S - 1            # 63 recurrence steps
NCORE = 8
VS = V // NCORE      # 4000 vocab rows per core
NTOK = T * B         # 2016 tokens
NTILE = 16           # token tiles of 128 (2048 padded)
NQ = 4               # vocab quarters of 1000 per tile
QV = VS // NQ        # 1000
EOS_IDX = 1
SHIFT = 20.0         # exp(logit - SHIFT) to avoid overflow


def _build_nc(eos_steps):
    """Build the SPMD bass program. eos_steps: sorted list of step indices t
    where any(next_words[t] == EOS_IDX)."""
    nc = bass.Bass(target_bir_lowering=False)
    has_eos = len(eos_steps) > 0
    n_eos = max(1, len(eos_steps))

    # ---------------- DRAM tensors ----------------
    xt_d = nc.dram_tensor("xt", [4, 128, 8, 512], BF, kind="ExternalInput")
    wih_d = nc.dram_tensor("wih", [128, 8, 3 * H], BF, kind="ExternalInput")
    whh_d = nc.dram_tensor("whh", [128, 4, 3 * H], BF, kind="ExternalInput")
    wis_d = nc.dram_tensor("wis", [128, 4, 3 * H], BF, kind="ExternalInput")
    whs_d = nc.dram_tensor("whs", [128, 4, 3 * H], BF, kind="ExternalInput")
    ow_d = nc.dram_tensor("ow", [128, 4, VS], BF, kind="ExternalInput")
    ob_d = nc.dram_tensor("ob", [1, VS], BF, kind="ExternalInput")
    h0_d = nc.dram_tensor("h0", [128, 4, B], F32, kind="ExternalInput")
    gib_d = nc.dram_tensor("gib", [128, 12], F32, kind="ExternalInput")
    bhn_d = nc.dram_tensor("bhn", [128, 4], F32, kind="ExternalInput")
    gisb_d = nc.dram_tensor("gisb", [128, 12], F32, kind="ExternalInput")
    bhsn_d = nc.dram_tensor("bhsn", [128, 4], F32, kind="ExternalInput")
    tq_d = nc.dram_tensor("tq", [128, NTILE, NQ], F32, kind="ExternalInput")
    eosm_d = nc.dram_tensor("eosm", [n_eos, 2, B], F32, kind="ExternalInput")

    se_o = nc.dram_tensor("se_out", [128, NTILE, NQ], F32, kind="ExternalOutput")
    tl_o = nc.dram_tensor("tl_out", [128, NTILE, NQ], F32, kind="ExternalOutput")


    with tile.TileContext(nc) as tc:
        with (
            tc.tile_pool(name="singles", bufs=1) as singles,
            tc.tile_pool(name="ew", bufs=3) as ew,
            tc.tile_pool(name="prjscr", bufs=2) as prjscr,
            tc.tile_pool(name="ps_rec", bufs=2, space="PSUM") as ps_rec,
            tc.tile_pool(name="ps_prj", bufs=2, space="PSUM") as ps_prj,
        ):
            # ---------------- Phase A: load persistent SBUF state ----------
            wih_sb = singles.tile([128, 8, 3 * H], BF)
            nc.sync.dma_start(wih_sb, wih_d[:, :, :])
            whh_sb = singles.tile([128, 4, 3 * H], BF)
            nc.sync.dma_start(whh_sb, whh_d[:, :, :])
            ow_sb = singles.tile([128, 4, VS], BF)
            nc.sync.dma_start(ow_sb, ow_d[:, :, :])

            brow_sb = singles.tile([128, VS], BF)
            nc.vector.memset(brow_sb, 0.0)
            nc.sync.dma_start(brow_sb[0:1, :], ob_d[:, :])
            onesrow_sb = singles.tile([128, 128], BF)
            nc.vector.memset(onesrow_sb, 0.0)
            nc.vector.memset(onesrow_sb[0:1, :], 1.0)

            gib_sb = singles.tile([128, 12], F32)
            nc.sync.dma_start(gib_sb, gib_d[:, :])
            bhn_sb = singles.tile([128, 4], F32)
            nc.sync.dma_start(bhn_sb, bhn_d[:, :])
            tq_sb = singles.tile([128, NTILE, NQ], F32)
            nc.sync.dma_start(tq_sb, tq_d[:, :, :])

            zero_sb = singles.tile([128, 1], F32)
            nc.vector.memset(zero_sb, 0.0)
            nc.const_aps.aps[(F32, 0.0)] = zero_sb
            shift_sb = singles.tile([128, 1], F32)
            nc.vector.memset(shift_sb, -SHIFT)

            iota_sb = singles.tile([128, QV], F32)
            nc.gpsimd.iota(iota_sb, [[1, QV]], channel_multiplier=0,
                           allow_small_or_imprecise_dtypes=True)

            h_sb = singles.tile([128, 4, B], F32)      # persistent word_h (h^T)
            nc.sync.dma_start(h_sb, h0_d[:, :, :])

            # word_h history (bf16, = h^T), step 63 is padding for tile 15
            wordh_sb = singles.tile([128, 4, T + 1, B], BF)
            nc.vector.memset(wordh_sb[:, :, T, :], 0.0)

            # gi for all steps, bf16: [p, gate_chunk, step, b]
            gi_sb = singles.tile([128, 12, T + 1, B], BF)

            seH_sb = singles.tile([128, NTILE, NQ], F32)
            tlH_sb = singles.tile([128, NTILE, NQ], F32)

            if has_eos:
                wis_sb = singles.tile([128, 4, 3 * H], BF)
                nc.sync.dma_start(wis_sb, wis_d[:, :, :])
                whs_sb = singles.tile([128, 4, 3 * H], BF)
                nc.sync.dma_start(whs_sb, whs_d[:, :, :])
                gisb_sb = singles.tile([128, 12], F32)
                nc.sync.dma_start(gisb_sb, gisb_d[:, :])
                bhsn_sb = singles.tile([128, 4], F32)
                nc.sync.dma_start(bhsn_sb, bhsn_d[:, :])
                sent_sb = singles.tile([128, 4, B], F32)
                nc.sync.dma_start(sent_sb, h0_d[:, :, :])
                sent_bf = singles.tile([128, 4, B], BF)
                nc.vector.tensor_copy(sent_bf, sent_sb)
                # eos masks, broadcast to 128 partitions: [128, n_eos, 2, B]
                eosm_sb = singles.tile([128, n_eos, 2, B], F32)
                eap = eosm_d[:, :, :]
                bcast = bass.AP(tensor=eap.tensor, offset=eap.offset,
                                ap=[[0, 128]] + [list(x) for x in eap.ap])
                nc.gpsimd.dma_start(out=eosm_sb, in_=bcast)

            # Barrier: all phase-A loads complete before compute. Keeps
            # per-instruction sync-wait counts at <=1 (TS-struct ISA limit).
            tc.strict_bb_all_engine_barrier()

            # ---------------- Phase B: gi for all tokens -------------------
            # gi[g, j] = sum_e wih^T[e, g] * x^T[e, j]  (+ per-gate bias)
            with (
                tc.tile_pool(name="ps_gi", bufs=2, space="PSUM") as ps_gi,
                tc.tile_pool(name="xts", bufs=1) as xts_pool,
            ):
                for tch in range(4):           # token chunks of 512 (last padded)
                    xt_sb = xts_pool.tile([128, 8, 512], BF, tag="xt")
                    nc.sync.dma_start(xt_sb, xt_d[tch])
                    for gc in range(12):       # gate chunks of 128
                        ps = ps_gi.tile([128, 512], F32, tag="gi")
                        for kc in range(8):
                            nc.tensor.matmul(
                                ps,
                                wih_sb[:, kc, gc * 128:(gc + 1) * 128],
                                xt_sb[:, kc, :],
                                start=(kc == 0), stop=(kc == 7))
                        nc.vector.tensor_scalar(
                            gi_sb[:, gc, tch * 16:(tch + 1) * 16, :],
                            ps.rearrange("p (s b) -> p s b", s=16),
                            gib_sb[:, gc:gc + 1], None, op0=OP.add)

            # ---------------- Phase C + D interleaved ----------------------
            hbf0 = ew.tile([128, 4, B], BF, tag="hbf_init")
            nc.vector.tensor_copy(hbf0, h_sb)
            h_bf_cur = hbf0

            # projection quarter emission schedule
            proj_q = [(i, q) for i in range(NTILE) for q in range(NQ)]
            proj_pos = 0

            def emit_proj_quarter(i, q):
                ps2 = ps_prj.tile([128, 2, 512], F32, tag="prj")
                for c2 in range(2):
                    vo = q * QV + c2 * 500
                    out_ap = ps2[:, c2, :500]
                    nc.tensor.matmul(out_ap, onesrow_sb,
                                     brow_sb[:, vo:vo + 500],
                                     start=True, stop=False)
                    for kc in range(4):
                        nc.tensor.matmul(
                            out_ap,
                            wordh_sb[:, kc, 4 * i:4 * i + 4, :],
                            ow_sb[:, kc, vo:vo + 500],
                            start=False, stop=(kc == 3))
                exp_scr = prjscr.tile([128, 2, 500], BF, tag="exp")
                nc.scalar.activation(
                    exp_scr, ps2[:, :, :500], AF.Exp, bias=shift_sb,
                    accum_out=seH_sb[:, i, q:q + 1])
                eq_scr = prjscr.tile([128, 2, 500], BF, tag="eq")
                nc.vector.scalar_tensor_tensor(
                    eq_scr,
                    iota_sb.rearrange("p (a v) -> p a v", a=2),
                    tq_sb[:, i, q:q + 1],
                    ps2[:, :, :500],
                    op0=OP.is_equal, op1=OP.mult,
                    accum_out=tlH_sb[:, i, q:q + 1])

            eos_set = {t: idx for idx, t in enumerate(eos_steps)}

            for t in range(T):
                # --- word GRU matmuls: pg[g, b] = sum_h whh^T[h,g] h^T[h,b]
                pg = ps_rec.tile([128, 12, B], F32, tag="pg")
                for gc in range(12):
                    for kc in range(4):
                        nc.tensor.matmul(
                            pg[:, gc, :],
                            whh_sb[:, kc, gc * 128:(gc + 1) * 128],
                            h_bf_cur[:, kc, :],
                            start=(kc == 0), stop=(kc == 3))

                # --- elementwise chain (sigma via tanh) ---
                # pre_rz = pg[rz] + gi[rz]   (biases already folded into gi)
                pre_rz = ew.tile([128, 8, B], F32, tag="pre_rz")
                nc.vector.tensor_add(pre_rz, pg[:, 0:8, :], gi_sb[:, 0:8, t, :])
                # tr = tanh(pre_r / 2); tz = tanh(pre_z / 2)
                tr = ew.tile([128, 4, B], F32, tag="tr")
                nc.scalar.activation(tr, pre_rz[:, 0:4, :], AF.Tanh, scale=0.5)
                tz = ew.tile([128, 4, B], F32, tag="tz")
                nc.scalar.activation(tz, pre_rz[:, 4:8, :], AF.Tanh, scale=0.5)
                # tmp_n = pg[n] + b_hh_n
                tmp_n = ew.tile([128, 4, B], F32, tag="tmp_n")
                nc.vector.tensor_tensor(
                    tmp_n, pg[:, 8:12, :],
                    bhn_sb[:, :, None].to_broadcast([128, 4, B]), OP.add)
                # pre_n = r * tmp_n + gi[n] = 0.5*(tr+1)*tmp_n + gi[n]
                nc.vector.scalar_tensor_tensor(
                    tmp_n, tr, 1.0, tmp_n, op0=OP.add, op1=OP.mult)
                nc.vector.scalar_tensor_tensor(
                    tmp_n, tmp_n, 0.5, gi_sb[:, 8:12, t, :],
                    op0=OP.mult, op1=OP.add)
                nn_t = ew.tile([128, 4, B], F32, tag="nn")
                nc.scalar.activation(nn_t, tmp_n, AF.Tanh)
                # h' = h + (1-z)*(n-h);  1-z = 0.5*(1-tz)
                # d = n - h ; s1 = (tz-1)*d ; h' = -0.5*s1 + h
                nc.vector.tensor_sub(nn_t, nn_t, h_sb)          # d
                s1 = ew.tile([128, 4, B], F32, tag="s1")
                nc.vector.scalar_tensor_tensor(
                    s1, tz, 1.0, nn_t, op0=OP.subtract, op1=OP.mult)
                nc.vector.scalar_tensor_tensor(
                    h_sb, s1, -0.5, h_sb, op0=OP.mult, op1=OP.add)

                # store word_h (pre-EOS-blend) as bf16; doubles as next mm input
                wh_slice = wordh_sb[:, :, t, :]
                nc.vector.tensor_copy(wh_slice, h_sb)
                h_bf_cur = wh_slice

                # --- rare: sent GRU + EOS blend ---
                if t in eos_set:
                    ei = eos_set[t]
                    pga = ps_rec.tile([128, 12, B], F32, tag="pg")
                    pgb = ps_rec.tile([128, 12, B], F32, tag="pg")
                    # rz gates: gi_s + gh_s accumulated together
                    for gc in range(8):
                        for kc in range(4):
                            nc.tensor.matmul(
                                pga[:, gc, :],
                                wis_sb[:, kc, gc * 128:(gc + 1) * 128],
                                h_bf_cur[:, kc, :],
                                start=(kc == 0), stop=False)
                        for kc in range(4):
                            nc.tensor.matmul(
                                pga[:, gc, :],
                                whs_sb[:, kc, gc * 128:(gc + 1) * 128],
                                sent_bf[:, kc, :],
                                start=False, stop=(kc == 3))
                    # n gates: keep gi_s (input part) and gh_s separate
                    for gc in range(8, 12):
                        for kc in range(4):
                            nc.tensor.matmul(
                                pga[:, gc, :],
                                wis_sb[:, kc, gc * 128:(gc + 1) * 128],
                                h_bf_cur[:, kc, :],
                                start=(kc == 0), stop=(kc == 3))
                        for kc in range(4):
                            nc.tensor.matmul(
                                pgb[:, gc - 8, :],
                                whs_sb[:, kc, gc * 128:(gc + 1) * 128],
                                sent_bf[:, kc, :],
                                start=(kc == 0), stop=(kc == 3))
                    pre_s = ew.tile([128, 8, B], F32, tag="pre_rz")
                    nc.vector.tensor_tensor(
                        pre_s, pga[:, 0:8, :],
                        gisb_sb[:, 0:8, None].to_broadcast([128, 8, B]),
                        OP.add)
                    trs = ew.tile([128, 4, B], F32, tag="tr")
                    nc.scalar.activation(trs, pre_s[:, 0:4, :], AF.Tanh,
                                         scale=0.5)
                    tzs = ew.tile([128, 4, B], F32, tag="tz")
                    nc.scalar.activation(tzs, pre_s[:, 4:8, :], AF.Tanh,
                                         scale=0.5)
                    tmps = ew.tile([128, 4, B], F32, tag="tmp_n")
                    nc.vector.tensor_tensor(
                        tmps, pgb[:, 0:4, :],
                        bhsn_sb[:, :, None].to_broadcast([128, 4, B]), OP.add)
                    nc.vector.scalar_tensor_tensor(
                        tmps, trs, 1.0, tmps, op0=OP.add, op1=OP.mult)
                    # + gi_s[n] part (pga chunks 8:12) then + b_ih_s[n]
                    nc.vector.scalar_tensor_tensor(
                        tmps, tmps, 0.5, pga[:, 8:12, :],
                        op0=OP.mult, op1=OP.add)
                    nc.vector.tensor_tensor(
                        tmps, tmps,
                        gisb_sb[:, 8:12, None].to_broadcast([128, 4, B]),
                        OP.add)
                    nns = ew.tile([128, 4, B], F32, tag="nn")
                    nc.scalar.activation(nns, tmps, AF.Tanh)
                    nc.vector.tensor_sub(nns, nns, sent_sb)     # d_s
                    s1s = ew.tile([128, 4, B], F32, tag="s1")
                    nc.vector.scalar_tensor_tensor(
                        s1s, tzs, 1.0, nns, op0=OP.subtract, op1=OP.mult)
                    tmpgru = ew.tile([128, 4, B], F32, tag="tmpgru")
                    nc.vector.scalar_tensor_tensor(
                        tmpgru, s1s, -0.5, sent_sb, op0=OP.mult, op1=OP.add)
                    # blends with mask m (um = 1-m)
                    m_b = eosm_sb[:, ei, 0, None, :].to_broadcast([128, 4, B])
                    um_b = eosm_sb[:, ei, 1, None, :].to_broadcast([128, 4, B])
                    ta = ew.tile([128, 4, B], F32, tag="ta")
                    tb = ew.tile([128, 4, B], F32, tag="tb")
                    nc.vector.tensor_mul(ta, h_sb, um_b)
                    nc.vector.tensor_mul(tb, tmpgru, m_b)
                    nc.vector.tensor_add(h_sb, ta, tb)
                    nc.vector.tensor_mul(ta, sent_sb, um_b)
                    nc.vector.tensor_add(sent_sb, ta, tb)
                    nc.vector.tensor_copy(sent_bf, sent_sb)
                    hbf2 = ew.tile([128, 4, B], BF, tag="hbf2")
                    nc.vector.tensor_copy(hbf2, h_sb)
                    h_bf_cur = hbf2

                # --- interleave one projection quarter when ready ---
                if proj_pos < len(proj_q):
                    i, q = proj_q[proj_pos]
                    if 4 * i + 3 <= t - 1:
                        emit_proj_quarter(i, q)
                        proj_pos += 1

            # remaining projection quarters
            while proj_pos < len(proj_q):
                emit_proj_quarter(*proj_q[proj_pos])
                proj_pos += 1

            # ---------------- outputs ----------------
            nc.sync.dma_start(se_o[:, :, :], seH_sb)
            nc.sync.dma_start(tl_o[:, :, :], tlH_sb)

    # Split multi-wait sync_infos into EventSemaphore carriers (ISA allows
    # at most one embedded wait per regular instruction).
    bass_rust.generate_event_semaphores(nc)
    return nc


def _prep_inputs(targets, targets_kws, sent_state, emb,
                 w_ih_w, w_hh_w, b_ih_w, b_hh_w,
                 w_ih_s, w_hh_s, b_ih_s, b_hh_s, out_w, out_b):
    """Host-side sharding/layout prep. Returns (common_map, per_core_maps,
    eos_steps, combine_info)."""
    tgt = np.asarray(targets)
    kws = np.asarray(targets_kws)
    lw = tgt[:, :-1].T            # (T, B) last words
    kw = kws[:, 1:].T             # (T, B) keywords
    nw = tgt[:, 1:].T             # (T, B) next words

    embT = np.ascontiguousarray(np.asarray(emb, np.float32).T)  # (512, V)
    kw_f = kw.reshape(-1).astype(np.int64)
    lw_f = lw.reshape(-1).astype(np.int64)
    xt_full = np.zeros((1024, 4 * 512), dtype=np.float32)
    xt_full[:512, :NTOK] = embT[:, kw_f]
    xt_full[512:, :NTOK] = embT[:, lw_f]
    # -> (4 chunks, 128 partitions, 8 kchunks, 512 tokens) bf16
    xt = np.ascontiguousarray(
        xt_full.reshape(8, 128, 4, 512).transpose(2, 1, 0, 3).astype(BF16NP))

    def wT(w):  # (G, K) -> (128, K//128, G) bf16 partition-major
        w = np.asarray(w, np.float32).T
        kk = w.shape[0] // 128
        return np.ascontiguousarray(
            w.reshape(kk, 128, w.shape[1]).transpose(1, 0, 2).astype(BF16NP))

    b_ih_w = np.asarray(b_ih_w, np.float32)
    b_hh_w = np.asarray(b_hh_w, np.float32)
    b_ih_s = np.asarray(b_ih_s, np.float32)
    b_hh_s = np.asarray(b_hh_s, np.float32)
    gib = np.concatenate([(b_ih_w + b_hh_w)[:1024], b_ih_w[1024:]])
    gisb = np.concatenate([(b_ih_s + b_hh_s)[:1024], b_ih_s[1024:]])

    def per_gate(v, nch):  # (nch*128,) -> (128, nch) [partition, chunk]
        return np.ascontiguousarray(
            v.reshape(nch, 128).T.astype(np.float32))

    h0 = np.asarray(sent_state, np.float32)[0]          # (B, H)
    h0T = np.ascontiguousarray(
        h0.T.reshape(4, 128, B).transpose(1, 0, 2))     # (128, 4, B)

    owT = np.asarray(out_w, np.float32).T               # (H, V)
    ob = np.asarray(out_b, np.float32)

    # eos schedule
    eos_rows = np.nonzero((nw == EOS_IDX).any(axis=1))[0]
    eos_steps = [int(t) for t in eos_rows]
    n_eos = max(1, len(eos_steps))
    eosm = np.zeros((n_eos, 2, B), np.float32)
    for i, t in enumerate(eos_steps):
        m = (nw[t] == EOS_IDX).astype(np.float32)
        eosm[i, 0] = m
        eosm[i, 1] = 1.0 - m

    common = {
        "xt": xt,
        "wih": wT(w_ih_w),
        "whh": wT(w_hh_w),
        "wis": wT(w_ih_s),
        "whs": wT(w_hh_s),
        "h0": h0T,
        "gib": per_gate(gib, 12),
        "bhn": per_gate(b_hh_w[1024:], 4),
        "gisb": per_gate(gisb, 12),
        "bhsn": per_gate(b_hh_s[1024:], 4),
        "eosm": eosm,
    }

    # per-core: vocab shard + target indices
    nw_f = nw.reshape(-1).astype(np.int64)              # token j = t*B + b
    per_core = []
    for c in range(NCORE):
        lo = c * VS
        ow_c = np.ascontiguousarray(
            owT[:, lo:lo + VS].reshape(4, 128, VS)
            .transpose(1, 0, 2).astype(BF16NP))
        ob_c = np.ascontiguousarray(
            ob[lo:lo + VS].reshape(1, VS).astype(BF16NP))
        tq = np.full((128, NTILE, NQ), -1.0, np.float32)
        loc = nw_f - lo
        ok = (loc >= 0) & (loc < VS)
        j = np.arange(NTOK)
        p = j % 128
        ti = j // 128
        qi = loc // QV
        qoff = loc - qi * QV
        for jj in np.nonzero(ok)[0]:
            tq[p[jj], ti[jj], qi[jj]] = float(qoff[jj])
        per_core.append({"ow": ow_c, "ob": ob_c, "tq": tq})

    info = {"nw_f": nw_f, "ob": ob}
    return common, per_core, eos_steps, info


def kernel(**inputs):
    targets = np.asarray(inputs["targets"])
    targets_len = np.asarray(inputs["targets_len"])
    common, per_core, eos_steps, info = _prep_inputs(
        targets, inputs["targets_kws"], inputs["sent_state"],
        inputs["emb"],
        inputs["w_ih_w"], inputs["w_hh_w"], inputs["b_ih_w"], inputs["b_hh_w"],
        inputs["w_ih_s"], inputs["w_hh_s"], inputs["b_ih_s"], inputs["b_hh_s"],
        inputs["out_w"], inputs["out_b"])

    nc = _build_nc(eos_steps)
    in_maps = [dict(common, **pc) for pc in per_core]
    try:
        res = run_bass_kernel_spmd(nc, in_maps, core_ids=list(range(NCORE)))
    except Exception:
        # transient device/relay errors: rebuild and retry once
        nc = _build_nc(eos_steps)
        res = run_bass_kernel_spmd(nc, in_maps, core_ids=list(range(NCORE)))
    results = res.results

    # ---- host combine (float64) ----
    se = np.zeros((128, NTILE, NQ), np.float64)
    tl = np.zeros((128, NTILE, NQ), np.float64)
    for r in results:
        se += r["se_out"].astype(np.float64)
        tl += r["tl_out"].astype(np.float64)
    se_tok = se.sum(axis=2).T.reshape(-1)[:NTOK]   # token j = tile*128 + p
    tl_tok = tl.sum(axis=2).T.reshape(-1)[:NTOK]
    lse = np.log(se_tok) + SHIFT
    tok_logp = (tl_tok - lse).reshape(T, B)
    valid = (targets_len[None, :].astype(np.int64)
             > np.arange(1, S)[:, None])
    loss = -(tok_logp * valid).sum()
    return np.float32(loss)

